# revision 43
# baseline (speedup 1.0000x reference)
"""Trainium2 Bass kernel for nn_MultiHeadODELinear.

Math: out = sum_{k=0..4} (t^k/k!) blockdiag(A_h)^k (x @ W.T + b)
The Taylor loop commutes with the token dimension, so it folds into the
projection:  out = x @ WT_eff + b_eff  with
  WT_eff = (E @ W).T,  b_eff = E @ b,  E = blockdiag(M_h),
  M_h  = sum_{k=0..4} (t^k/k!) A_h^k   (16 heads of 64x64).

W_eff / b_eff are tiny (1Kx1K) and are computed on the host in float64 as
part of input preparation (kernel() also casts x to fp16 host-side); the
device kernel is a pure fp16 GEMM.

Per-core (data-parallel over batch, one [4096, 1024] batch per core):
  - x arrives pre-transposed via the DMA XBAR (InstDmaTransposeAnt, fp16):
    out[p, m, s] = x[s, 128*m + p] -- d lands on partitions with zero PE
    work.  One transpose DMA per 4-tile group (batched: the framework
    recycles 8 hw DMA queues depth-1, so many small DMAs serialize on
    issue latency; 8 transposes + 8 out-DMAs + 3 const DMAs per pass).
  - per 128-token tile: 16 accumulating fp16 matmuls (1 cyc/row, moving
    dim 512) into 2 PSUM banks; bias-add on the PSUM->SBUF copyback (DVE;
    Pool cannot read PSUM on TRN2); fp16 out, host upcasts to f32.
  - a ~7us PE warmup of junk matmuls covers the input-DMA ramp so the
    stream starts at full clock (PE runs at reduced clock for ~3us after
    any idle); group 0 runs two-phase across all 8 PSUM banks so it only
    needs weight chunks 0-3 + the first transpose half to start.
  - input DMAs ride the SP queue in exact prologue order; out-DMAs ride
    the Act queue so their wait-for-copyback cannot head-of-line-block
    the transpose stream.

PE does nothing but the 512 x 512-row matmuls: 262144 rows x 0.4167 ns =
109.2 us/core = the tensor-engine roofline for this GEMM; DMA (x 8 MiB +
out 8 MiB + W 2 MiB = 18 MiB/core) hides under it.  The last group's
oh=1 PSUM banks are pre-biased (DVE write + start=False accumulate) so
the final copybacks run DVE || Act, shortening the drain to ~3.5 us.
Measured (NTFF device profile, 8 cores): ~135 us single-shot (preamble
~6 + DMA ramp ~11 + stream ~110 + drain ~3.5) vs ~176 us for the
previous f32r PE-transpose kernel; rel err vs the f32 reference 3.3e-4.
Ramp notes from HW traces: the DMA complex serves one DMA at a time in
global issue order (queue-splitting or any DMA ahead of the weights
delays the critical path), and the first XBAR transpose pays ~2 us of
setup -- the ~17.5 us to stream start is DMA-latency-bound.
"""

import sys

for _p in ("/opt/trn_rl_repo",):
    if _p not in sys.path:
        sys.path.insert(0, _p)

import numpy as np

import concourse.bass as bass  # noqa: F401
import concourse.tile as tile
from concourse import bacc, mybir
from concourse import bass_utils

F32 = mybir.dt.float32
F16 = mybir.dt.float16

B, S, D = 8, 4096, 1024
H, HD = 16, 64
ORDERS = 4
P = 128
NCHUNK = D // P          # 8 chunks of 128 along the 1024 contraction dim
TTILES = S // P          # 32 token tiles per core
N_CORES = 8

_NC_CACHE = {}


def _build_nc(repeats=1, variant=()):
    variant = set(variant)
    nc = bacc.Bacc("TRN2", target_bir_lowering=False, debug=False)

    x_d = nc.dram_tensor("x", [S, D], F16, kind="ExternalInput").ap()
    w_d = nc.dram_tensor("w", [P, NCHUNK, D], F16, kind="ExternalInput").ap()
    bb_d = nc.dram_tensor("bb", [P, D], F16, kind="ExternalInput").ap()
    o_d = nc.dram_tensor("out", [S, D], F16, kind="ExternalOutput").ap()

    GRP = 4                       # token tiles per DMA group
    NGRP = TTILES // GRP          # 8 groups per pass
    n_grps = NGRP * repeats
    LA = 2                        # group lookahead (8 tiles ahead of PE)

    # out rows of group g are (j*128 + p) for j in 0..3 -> [p, j, o] SBUF view
    o_v = o_d.rearrange("(g j p) o -> g p j o", p=P, j=GRP)

    NWARM = 0 if "nowarm" in variant else 34

    with tile.TileContext(nc) as tc:
        with tc.tile_pool(name="const", bufs=1) as const_pool, \
             tc.tile_pool(name="xt", bufs=LA + 2) as xt_pool, \
             tc.tile_pool(name="osb", bufs=3) as o_pool, \
             tc.tile_pool(name="ps_o", bufs=4, space="PSUM") as ps_o:

            wte = const_pool.tile([P, NCHUNK, D], F16)
            b_bcast = const_pool.tile([P, D], F16)

            # PE warmup: junk matmuls keep the PE busy through the input
            # DMA ramp so the real stream starts at full clock (matmul speed
            # is roughly halved for ~3us after any PE idle while the clock
            # re-ramps). Accumulates into a main-pool PSUM slot (ring WAR
            # keeps it safe) so all 8 banks stay available to the stream.
            if NWARM and n_grps > 0:
                warm = const_pool.tile([P, 512], F16, name="warm")
                nc.gpsimd.memset(warm[:], 0.0)
                wps = ps_o.tile([P, 512], F32, tag="ps_out0", name="wps")
                for i in range(NWARM):
                    nc.tensor.matmul(wps[:], warm[:, 0:P], warm[:],
                                     start=(i == 0), stop=(i == NWARM - 1))

            def stage_a(g, split=False):
                gg = g % NGRP
                xt = xt_pool.tile([P, NCHUNK, GRP * P], F16, name="xt")
                r0 = gg * GRP * P
                if split:
                    # two 2-tile halves so the first matmuls start sooner
                    h = GRP * P // 2
                    nc.sync.dma_start_transpose(
                        xt[:, :, 0:h], x_d[r0:r0 + h, :])
                    nc.sync.dma_start_transpose(
                        xt[:, :, h:2 * h], x_d[r0 + h:r0 + 2 * h, :])
                else:
                    nc.sync.dma_start_transpose(
                        xt[:], x_d[r0:r0 + GRP * P, :])
                return xt

            def mm_tile(ps, xt, j, m_lo, m_hi, prebias=False):
                for m in range(m_lo, m_hi):
                    for oh in range(2):
                        pb = prebias and oh == 1
                        nc.tensor.matmul(
                            ps[oh][:], xt[:, m, j * P:(j + 1) * P],
                            wte[:, m, oh * 512:(oh + 1) * 512],
                            start=(m == 0 and not pb),
                            stop=(m == NCHUNK - 1),
                            skip_group_check=pb)

            def copyback(o_sb, ps, j):
                # both halves on DVE: GPSIMD/Pool cannot read PSUM on TRN2
                nc.vector.tensor_tensor(o_sb[:, j, 0:512], ps[0][:],
                                        b_bcast[:, 0:512],
                                        mybir.AluOpType.add)
                nc.vector.tensor_tensor(o_sb[:, j, 512:1024], ps[1][:],
                                        b_bcast[:, 512:1024],
                                        mybir.AluOpType.add)

            def new_ps():
                return [ps_o.tile([P, 512], F32, tag=f"ps_out{oh}",
                                  name=f"ps_out{oh}")
                        for oh in range(2)]

            def stage_b(g, xt):
                gg = g % NGRP
                o_sb = o_pool.tile([P, GRP, D], F16, name="o_sb")
                if g == 0:
                    # group 0: two-phase over all 4 tiles (chunks 0-3 then
                    # 4-7, 8 PSUM banks) so the stream starts as soon as
                    # weight chunks 0-3 + the first transpose half land and
                    # never waits on the second weight half.
                    ps4 = [new_ps() for _ in range(GRP)]
                    for j in range(GRP):
                        mm_tile(ps4[j], xt, j, 0, 4)
                    for j in range(GRP):
                        mm_tile(ps4[j], xt, j, 4, NCHUNK)
                        copyback(o_sb, ps4[j], j)
                    rest = ()
                else:
                    rest = range(GRP)
                last = (g == n_grps - 1)
                for j in rest:
                    ps = new_ps()
                    if last:
                        # pre-bias the oh=1 bank (DVE write, off the PE
                        # path; its chain starts with start=False) so the
                        # final copybacks run DVE || Act instead of two
                        # serial DVE adds -- shortens the drain.
                        nc.vector.tensor_copy(ps[1][:], b_bcast[:, 512:1024])
                    mm_tile(ps, xt, j, 0, NCHUNK, prebias=last)
                    if last:
                        nc.vector.tensor_tensor(o_sb[:, j, 0:512], ps[0][:],
                                                b_bcast[:, 0:512],
                                                mybir.AluOpType.add)
                        nc.scalar.mul(o_sb[:, j, 512:1024], ps[1][:], 1.0)
                    else:
                        copyback(o_sb, ps, j)
                # out DMA on the Act queue: its wait-for-copyback would
                # otherwise head-of-line-block the next xt transpose on SP.
                # Last group drains per-tile so the final DMA is small.
                if g == n_grps - 1:
                    for j in range(GRP):
                        nc.scalar.dma_start(o_v[gg, :, j:j + 1, :],
                                            o_sb[:, j:j + 1, :])
                else:
                    nc.scalar.dma_start(o_v[gg], o_sb[:])

            from collections import deque
            q = deque()
            # Prologue, all on the SP queue so the transfer order on the
            # serial DMA complex is exactly as written; the PE warmup above
            # covers this ramp so the stream starts at full clock. Weight
            # halves bracket the first transpose half: group 0's interleaved
            # tile order only needs chunks 0-3 for its first 16 matmuls.
            # Transposes on the SP HWDGE queue; weights/bias via the GpSimd
            # SWDGE path, which generates descriptors in software and may be
            # served concurrently with the HWDGE stream (HWDGE queues are
            # served one DMA at a time in global issue order, so ordering
            # within HWDGE cannot overlap the ramp).
            nc.gpsimd.dma_start(wte[:, 0:4, :], w_d[:, 0:4, :])
            if n_grps > 0:
                xt0 = xt_pool.tile([P, NCHUNK, GRP * P], F16, name="xt")
                h = GRP * P // 2
                nc.sync.dma_start_transpose(xt0[:, :, 0:h], x_d[0:h, :])
                nc.sync.dma_start_transpose(xt0[:, :, h:2 * h],
                                            x_d[h:2 * h, :])
                q.append(xt0)
            nc.gpsimd.dma_start(wte[:, 4:8, :], w_d[:, 4:8, :])
            nc.gpsimd.dma_start(b_bcast[:], bb_d[:])

            for g in range(1, min(LA, n_grps)):
                q.append(stage_a(g, split=True))
            for g in range(n_grps):
                if g + LA < n_grps:
                    q.append(stage_a(g + LA))
                stage_b(g, q.popleft())

    nc.compile()
    return nc


def get_nc(repeats=1, variant=()):
    key = (repeats, tuple(variant))
    if key not in _NC_CACHE:
        _NC_CACHE[key] = _build_nc(repeats, variant)
    return _NC_CACHE[key]


def _host_fold(t_scalar, W, b, A):
    """W_eff/b_eff of the folded Taylor series, in float64 on host."""
    t = float(np.asarray(t_scalar, dtype=np.float64))
    A64 = np.asarray(A, dtype=np.float64)          # [H, HD, HD]
    M = np.broadcast_to(np.eye(HD), (H, HD, HD)).copy()
    term = M.copy()
    for k in range(1, ORDERS + 1):
        term = (t / k) * np.einsum("hij,hjk->hik", A64, term)
        M += term
    W64 = np.asarray(W, dtype=np.float64).reshape(H, HD, D)
    W_eff = np.einsum("hij,hjd->hid", M, W64).reshape(D, D)   # [o, d]
    b_eff = np.einsum("hij,hj->hi", M,
                      np.asarray(b, dtype=np.float64).reshape(H, HD))
    return W_eff, b_eff.reshape(D)


def make_in_maps(x, t_scalar, W, b, A):
    W_eff, b_eff = _host_fold(t_scalar, W, b, A)
    # xt chunk layout from the XBAR transpose: xt[p, m, s] = x[s, 128m + p],
    # so the weight SBUF tile needs wte[p, m, o] = WT_eff[128m + p, o].
    WT = np.ascontiguousarray(W_eff.T)             # [d, o]
    wte_np = np.ascontiguousarray(
        WT.reshape(NCHUNK, P, D).transpose(1, 0, 2)).astype(np.float16)
    bb_np = np.ascontiguousarray(
        np.broadcast_to(b_eff.astype(np.float16), (P, D)))
    x16 = np.asarray(x, dtype=np.float16)
    return [{"x": np.ascontiguousarray(x16[i]), "w": wte_np, "bb": bb_np}
            for i in range(N_CORES)]


def kernel(x, t_scalar, W, b, A):
    nc = get_nc()
    in_maps = make_in_maps(x, t_scalar, W, b, A)
    res = bass_utils.run_bass_kernel_spmd(nc, in_maps,
                                          core_ids=list(range(N_CORES)))
    return np.stack([res.results[i]["out"].astype(np.float32)
                     for i in range(N_CORES)], axis=0)


if __name__ == "__main__":
    rng = np.random.default_rng(0)
    x = rng.standard_normal((B, S, D), dtype=np.float32)
    W = rng.standard_normal((D, D), dtype=np.float32) / 32.0
    b = rng.standard_normal((D,), dtype=np.float32) * 0.01
    A = rng.standard_normal((H, HD, HD), dtype=np.float32) * 0.02
    t = np.float32(0.6)
    out = kernel(x, t, W, b, A)
    print("out", out.shape, out.dtype)


# revision 44
# speedup vs baseline: 1.0613x; 1.0613x over previous
"""Trainium2 Bass kernel for nn_MultiHeadODELinear.

Math: out = sum_{k=0..4} (t^k/k!) blockdiag(A_h)^k (x @ W.T + b)
The Taylor loop commutes with the token dimension, so it folds into the
projection:  out = x @ WT_eff + b_eff  with
  WT_eff = (E @ W).T,  b_eff = E @ b,  E = blockdiag(M_h),
  M_h  = sum_{k=0..4} (t^k/k!) A_h^k   (16 heads of 64x64).

W_eff / b_eff are tiny (1Kx1K) and are computed on the host in float64 as
part of input preparation (kernel() also casts x to fp16 host-side); the
device kernel is a pure fp16 GEMM.

Per-core (data-parallel over batch, one [4096, 1024] batch per core):
  - x arrives pre-transposed via the DMA XBAR (InstDmaTransposeAnt, fp16):
    out[p, m, s] = x[s, 128*m + p] -- d lands on partitions with zero PE
    work.  One transpose DMA per 4-tile group (batched: the framework
    recycles 8 hw DMA queues depth-1, so many small DMAs serialize on
    issue latency; 8 transposes + 8 out-DMAs + 3 const DMAs per pass).
  - per 128-token tile: 16 accumulating fp16 matmuls (1 cyc/row, moving
    dim 512) into 2 PSUM banks; bias-add on the PSUM->SBUF copyback (DVE;
    Pool cannot read PSUM on TRN2); fp16 out, host upcasts to f32.
  - a ~7us PE warmup of junk matmuls covers the input-DMA ramp so the
    stream starts at full clock (PE runs at reduced clock for ~3us after
    any idle); group 0 runs two-phase across all 8 PSUM banks so it only
    needs weight chunks 0-3 + the first transpose half to start.
  - input DMAs ride the SP queue in exact prologue order; out-DMAs ride
    the Act queue so their wait-for-copyback cannot head-of-line-block
    the transpose stream.

PE does nothing but the 512 x 512-row matmuls: 262144 rows x 0.4167 ns =
109.2 us/core = the tensor-engine roofline for this GEMM; DMA (x 8 MiB +
out 8 MiB + W 2 MiB = 18 MiB/core) hides under it.  The last group's
oh=1 PSUM banks are pre-biased (DVE write + start=False accumulate) so
the final copybacks run DVE || Act, shortening the drain to ~3.5 us.
Measured (NTFF device profile, 8 cores): ~135 us single-shot (preamble
~6 + DMA ramp ~11 + stream ~110 + drain ~3.5) vs ~176 us for the
previous f32r PE-transpose kernel; rel err vs the f32 reference 3.3e-4.
Ramp notes from HW traces: the DMA complex serves one DMA at a time in
global issue order (queue-splitting or any DMA ahead of the weights
delays the critical path), and the first XBAR transpose pays ~2 us of
setup -- the ~17.5 us to stream start is DMA-latency-bound.
"""

import sys

for _p in ("/opt/trn_rl_repo",):
    if _p not in sys.path:
        sys.path.insert(0, _p)

import numpy as np

import concourse.bass as bass  # noqa: F401
import concourse.tile as tile
from concourse import bacc, mybir
from concourse import bass_utils

F32 = mybir.dt.float32
F16 = mybir.dt.float16

B, S, D = 8, 4096, 1024
H, HD = 16, 64
ORDERS = 4
P = 128
NCHUNK = D // P          # 8 chunks of 128 along the 1024 contraction dim
TTILES = S // P          # 32 token tiles per core
N_CORES = 8

_NC_CACHE = {}


def _build_nc(repeats=1, variant=()):
    variant = set(variant)
    nc = bacc.Bacc("TRN2", target_bir_lowering=False, debug=False)

    x_d = nc.dram_tensor("x", [S, D], F16, kind="ExternalInput").ap()
    w_d = nc.dram_tensor("w", [P, NCHUNK, D], F16, kind="ExternalInput").ap()
    bb_d = nc.dram_tensor("bb", [P, D], F16, kind="ExternalInput").ap()
    o_d = nc.dram_tensor("out", [S, D], F16, kind="ExternalOutput").ap()

    GRP = 4                       # token tiles per DMA group
    NGRP = TTILES // GRP          # 8 groups per pass
    n_grps = NGRP * repeats
    LA = 2                        # group lookahead (8 tiles ahead of PE)

    # out rows of group g are (j*128 + p) for j in 0..3 -> [p, j, o] SBUF view
    o_v = o_d.rearrange("(g j p) o -> g p j o", p=P, j=GRP)

    NWARM = 0 if "nowarm" in variant else 34

    with tile.TileContext(nc) as tc:
        with tc.tile_pool(name="const", bufs=1) as const_pool, \
             tc.tile_pool(name="xt", bufs=LA + 2) as xt_pool, \
             tc.tile_pool(name="osb", bufs=3) as o_pool, \
             tc.tile_pool(name="ps_o", bufs=4, space="PSUM") as ps_o:

            wte = const_pool.tile([P, NCHUNK, D], F16)
            b_bcast = const_pool.tile([P, D], F16)

            # PE warmup: junk matmuls keep the PE busy through the input
            # DMA ramp so the real stream starts at full clock (matmul speed
            # is roughly halved for ~3us after any PE idle while the clock
            # re-ramps). Accumulates into a main-pool PSUM slot (ring WAR
            # keeps it safe) so all 8 banks stay available to the stream.
            if NWARM and n_grps > 0:
                warm = const_pool.tile([P, 512], F16, name="warm")
                nc.gpsimd.memset(warm[:], 0.0)
                wps = ps_o.tile([P, 512], F32, tag="ps_out0", name="wps")
                for i in range(NWARM):
                    nc.tensor.matmul(wps[:], warm[:, 0:P], warm[:],
                                     start=(i == 0), stop=(i == NWARM - 1))

            def stage_a(g, split=False):
                gg = g % NGRP
                xt = xt_pool.tile([P, NCHUNK, GRP * P], F16, name="xt")
                r0 = gg * GRP * P
                if split:
                    # two 2-tile halves so the first matmuls start sooner
                    h = GRP * P // 2
                    nc.sync.dma_start_transpose(
                        xt[:, :, 0:h], x_d[r0:r0 + h, :])
                    nc.sync.dma_start_transpose(
                        xt[:, :, h:2 * h], x_d[r0 + h:r0 + 2 * h, :])
                else:
                    nc.sync.dma_start_transpose(
                        xt[:], x_d[r0:r0 + GRP * P, :])
                return xt

            def mm_tile(ps, xt, j, m_lo, m_hi, prebias=False):
                for m in range(m_lo, m_hi):
                    for oh in range(2):
                        pb = prebias and oh == 1
                        nc.tensor.matmul(
                            ps[oh][:], xt[:, m, j * P:(j + 1) * P],
                            wte[:, m, oh * 512:(oh + 1) * 512],
                            start=(m == 0 and not pb),
                            stop=(m == NCHUNK - 1),
                            skip_group_check=pb)

            def copyback(o_sb, ps, j):
                # both halves on DVE: GPSIMD/Pool cannot read PSUM on TRN2
                nc.vector.tensor_tensor(o_sb[:, j, 0:512], ps[0][:],
                                        b_bcast[:, 0:512],
                                        mybir.AluOpType.add)
                nc.vector.tensor_tensor(o_sb[:, j, 512:1024], ps[1][:],
                                        b_bcast[:, 512:1024],
                                        mybir.AluOpType.add)

            def new_ps():
                return [ps_o.tile([P, 512], F32, tag=f"ps_out{oh}",
                                  name=f"ps_out{oh}")
                        for oh in range(2)]

            def stage_b(g, xt):
                gg = g % NGRP
                o_sb = o_pool.tile([P, GRP, D], F16, name="o_sb")
                if g == 0:
                    # group 0: two-phase over all 4 tiles (chunks 0-3 then
                    # 4-7, 8 PSUM banks) so the stream starts as soon as
                    # weight chunks 0-3 + the first transpose half land and
                    # never waits on the second weight half.
                    ps4 = [new_ps() for _ in range(GRP)]
                    for j in range(GRP):
                        mm_tile(ps4[j], xt, j, 0, 4)
                    for j in range(GRP):
                        mm_tile(ps4[j], xt, j, 4, NCHUNK)
                        copyback(o_sb, ps4[j], j)
                    rest = ()
                else:
                    rest = range(GRP)
                last = (g == n_grps - 1)
                for j in rest:
                    ps = new_ps()
                    if last:
                        # pre-bias the oh=1 bank (DVE write, off the PE
                        # path; its chain starts with start=False) so the
                        # final copybacks run DVE || Act instead of two
                        # serial DVE adds -- shortens the drain.
                        nc.vector.tensor_copy(ps[1][:], b_bcast[:, 512:1024])
                    mm_tile(ps, xt, j, 0, NCHUNK, prebias=last)
                    if last:
                        nc.vector.tensor_tensor(o_sb[:, j, 0:512], ps[0][:],
                                                b_bcast[:, 0:512],
                                                mybir.AluOpType.add)
                        nc.scalar.mul(o_sb[:, j, 512:1024], ps[1][:], 1.0)
                    else:
                        copyback(o_sb, ps, j)
                # out DMA on the Act queue: its wait-for-copyback would
                # otherwise head-of-line-block the next xt transpose on SP.
                # Last group drains per-tile so the final DMA is small.
                if g == n_grps - 1:
                    for j in range(GRP):
                        nc.scalar.dma_start(o_v[gg, :, j:j + 1, :],
                                            o_sb[:, j:j + 1, :])
                else:
                    nc.scalar.dma_start(o_v[gg], o_sb[:])

            from collections import deque
            q = deque()
            # Prologue, all on the SP queue so the transfer order on the
            # serial DMA complex is exactly as written; the PE warmup above
            # covers this ramp so the stream starts at full clock. Weight
            # halves bracket the first transpose half: group 0's interleaved
            # tile order only needs chunks 0-3 for its first 16 matmuls.
            # All input DMAs on the SP queue: the DMA complex serves one
            # DMA at a time in global issue order, so queue-splitting (or
            # any extra DMA ahead of the weights) only delays the critical
            # path -- both measured worse.
            nc.sync.dma_start(wte[:, 0:4, :], w_d[:, 0:4, :])
            if n_grps > 0:
                xt0 = xt_pool.tile([P, NCHUNK, GRP * P], F16, name="xt")
                h = GRP * P // 2
                nc.sync.dma_start_transpose(xt0[:, :, 0:h], x_d[0:h, :])
                nc.sync.dma_start_transpose(xt0[:, :, h:2 * h],
                                            x_d[h:2 * h, :])
                q.append(xt0)
            nc.sync.dma_start(wte[:, 4:8, :], w_d[:, 4:8, :])
            nc.sync.dma_start(b_bcast[:], bb_d[:])

            for g in range(1, min(LA, n_grps)):
                q.append(stage_a(g, split=True))
            for g in range(n_grps):
                if g + LA < n_grps:
                    q.append(stage_a(g + LA))
                stage_b(g, q.popleft())

    nc.compile()
    return nc


def get_nc(repeats=1, variant=()):
    key = (repeats, tuple(variant))
    if key not in _NC_CACHE:
        _NC_CACHE[key] = _build_nc(repeats, variant)
    return _NC_CACHE[key]


def _host_fold(t_scalar, W, b, A):
    """W_eff/b_eff of the folded Taylor series, in float64 on host."""
    t = float(np.asarray(t_scalar, dtype=np.float64))
    A64 = np.asarray(A, dtype=np.float64)          # [H, HD, HD]
    M = np.broadcast_to(np.eye(HD), (H, HD, HD)).copy()
    term = M.copy()
    for k in range(1, ORDERS + 1):
        term = (t / k) * np.einsum("hij,hjk->hik", A64, term)
        M += term
    W64 = np.asarray(W, dtype=np.float64).reshape(H, HD, D)
    W_eff = np.einsum("hij,hjd->hid", M, W64).reshape(D, D)   # [o, d]
    b_eff = np.einsum("hij,hj->hi", M,
                      np.asarray(b, dtype=np.float64).reshape(H, HD))
    return W_eff, b_eff.reshape(D)


def make_in_maps(x, t_scalar, W, b, A):
    W_eff, b_eff = _host_fold(t_scalar, W, b, A)
    # xt chunk layout from the XBAR transpose: xt[p, m, s] = x[s, 128m + p],
    # so the weight SBUF tile needs wte[p, m, o] = WT_eff[128m + p, o].
    WT = np.ascontiguousarray(W_eff.T)             # [d, o]
    wte_np = np.ascontiguousarray(
        WT.reshape(NCHUNK, P, D).transpose(1, 0, 2)).astype(np.float16)
    bb_np = np.ascontiguousarray(
        np.broadcast_to(b_eff.astype(np.float16), (P, D)))
    x16 = np.asarray(x, dtype=np.float16)
    return [{"x": np.ascontiguousarray(x16[i]), "w": wte_np, "bb": bb_np}
            for i in range(N_CORES)]


def kernel(x, t_scalar, W, b, A):
    nc = get_nc()
    in_maps = make_in_maps(x, t_scalar, W, b, A)
    res = bass_utils.run_bass_kernel_spmd(nc, in_maps,
                                          core_ids=list(range(N_CORES)))
    return np.stack([res.results[i]["out"].astype(np.float32)
                     for i in range(N_CORES)], axis=0)


if __name__ == "__main__":
    rng = np.random.default_rng(0)
    x = rng.standard_normal((B, S, D), dtype=np.float32)
    W = rng.standard_normal((D, D), dtype=np.float32) / 32.0
    b = rng.standard_normal((D,), dtype=np.float32) * 0.01
    A = rng.standard_normal((H, HD, HD), dtype=np.float32) * 0.02
    t = np.float32(0.6)
    out = kernel(x, t, W, b, A)
    print("out", out.shape, out.dtype)
